# revision 1
# baseline (speedup 1.0000x reference)
"""Bidirectional masked LSTM encoder (B=512, T=1024, EMB=HID=64) on 8 TRN2 cores.

Strategy: data-parallel over batch (64 samples/core). Per core, the forward and
backward LSTM streams run as two independent instruction streams that the Tile
scheduler staggers to hide per-step latency.

Per direction/step:
  zq[128,128] (PSUM) = [i;f | o;2g] gates via 4 matmuls:
     x-part: K=65 (emb row + mask-indicator row as bias multiplier), start=True
     h-part: K=64, accumulate
  S = sigmoid(zq)                       (one ACT op; tanh(g)=2*sigmoid(2g)-1)
  u = f*c (DVE); v = i*gp (DVE); t2 = 2v+u (gpsimd); cn = t2-i (gpsimd)
  tc = tanh(cn) (ACT); p = o*tc (gpsimd)
  state[c|h] <- copy_predicated(mask, [cn|p])   (DVE; Keras mask_zero carry)

Embedding gather: gpsimd ap_gather from SBUF-resident table [128, 1000]
(rows 0:64 emb.T, rows 64:128 = (token!=0) indicator), 64-step chunks.
"""

import numpy as np

VOCAB = 1000
EMB = 64
HID = 64
B_FULL = 512
T_FULL = 1024
N_CORES = 8
B = B_FULL // N_CORES  # 64 per core

_COMPILED = {}


# ----------------------------------------------------------------------------
# Host-side input packing
# ----------------------------------------------------------------------------

def _pack_wrapped_idx(jw: np.ndarray) -> np.ndarray:
    """Pack a flat index stream into ap_gather's wrapped layout.

    Index j lives at partition j%16, free slot j//16, replicated into each of
    the eight 16-partition blocks (one per gpsimd core).
    """
    n = jw.shape[0]
    assert n % 16 == 0
    w = jw.reshape(n // 16, 16).T.astype(np.int16)  # [16, n/16]
    return np.tile(w, (8, 1))  # [128, n/16]


def _host_prep_shared(emb_table, Wx_f, Wh_f, b_f, Wx_b, Wh_b, b_b):
    """Weight/table tensors shared by all cores."""
    f32 = np.float32

    def packs(Wx, Wh, b):
        # gate order in reference: z -> i, f, g, o (cols 0:64,64:128,128:192,192:256)
        # pair "if" = (i, f) cols 0:128; pair "og" = (o, 2*g).
        lx_if = np.vstack([Wx[:, 0:128], b[None, 0:128]]).astype(f32)          # [65,128]
        og_w = np.hstack([2.0 * Wx[:, 128:192], Wx[:, 192:256]])
        og_b = np.concatenate([2.0 * b[128:192], b[192:256]])
        lx_og = np.vstack([og_w, og_b[None, :]]).astype(f32)                   # [65,128]
        lh_if = Wh[:, 0:128].astype(f32)                                       # [64,128]
        lh_og = np.hstack([2.0 * Wh[:, 128:192], Wh[:, 192:256]]).astype(f32)  # [64,128]
        return lx_if, lx_og, lh_if, lh_og

    lx_if_f, lx_og_f, lh_if_f, lh_og_f = packs(Wx_f, Wh_f, b_f)
    lx_if_b, lx_og_b, lh_if_b, lh_og_b = packs(Wx_b, Wh_b, b_b)

    ind = (np.arange(VOCAB) != 0).astype(f32)
    tab = np.vstack([emb_table.T.astype(f32), np.tile(ind[None, :], (64, 1))])  # [128,1000]

    return {
        "tab": tab,
        "lx_if_f": lx_if_f, "lx_og_f": lx_og_f, "lx_if_b": lx_if_b, "lx_og_b": lx_og_b,
        "lh_if_f": lh_if_f, "lh_og_f": lh_og_f, "lh_if_b": lh_if_b, "lh_og_b": lh_og_b,
    }


def _host_prep_core(tok_c: np.ndarray, T: int) -> np.ndarray:
    """Per-core gather-index stream: fwd (t ascending) then bwd (t descending)."""
    jw_f = tok_c.T.reshape(-1)                # j = t*B + b
    jw_b = tok_c[:, ::-1].T.reshape(-1)
    return np.concatenate([_pack_wrapped_idx(jw_f), _pack_wrapped_idx(jw_b)], axis=1)


# ----------------------------------------------------------------------------
# Device program
# ----------------------------------------------------------------------------

def _build_body(tc, outs, ins, T: int, knobs=None):
    import concourse.bass as bass
    from concourse import mybir

    f32 = mybir.dt.float32
    Sig = mybir.ActivationFunctionType.Sigmoid
    Tanh = mybir.ActivationFunctionType.Tanh
    Op = mybir.AluOpType

    from contextlib import ExitStack

    nc = tc.nc
    TC = 64                      # steps per gather chunk
    NCH = T // TC                # chunks per stream
    out = outs["out"]

    stack = ExitStack()
    def pool(name, bufs, **kw):
        return stack.enter_context(tc.tile_pool(name=name, bufs=bufs, **kw))

    kn = {"gbuf": 4, "zq": 6, "sg": 4, "work": 3}
    kn.update(knobs or {})
    consts = pool("consts", 1)
    gpool = pool("gbuf", kn["gbuf"])
    zqpool = pool("zq", kn["zq"], space="PSUM")
    spool = pool("sg", kn["sg"])
    work = pool("work", kn["work"])
    stpool = pool("state", 1)

    # --- constants into SBUF
    tab = consts.tile([128, VOCAB], f32)
    nc.sync.dma_start(out=tab, in_=ins["tab"])
    idx = consts.tile([128, 2 * T * B // 16], mybir.dt.int16)
    nc.sync.dma_start(out=idx, in_=ins["idx16"])

    W = {}
    for d in ("f", "b"):
        for p_ in ("if", "og"):
            wx = consts.tile([65, 128], f32, tag=f"lx_{p_}_{d}")
            nc.sync.dma_start(out=wx, in_=ins[f"lx_{p_}_{d}"])
            wh_t = consts.tile([128, 128], f32, tag=f"lh_{p_}_{d}")
            nc.sync.dma_start(out=wh_t[64:128, :], in_=ins[f"lh_{p_}_{d}"])
            W[f"x_{p_}_{d}"] = wx
            W[f"h_{p_}_{d}"] = wh_t[64:128, :]

    # --- per-stream persistent state: [64, 132] = (c @ 0:64 | h @ 68:132).
    # The 68-stride keeps the (c|h) pair non-contiguous so the combined
    # predicated state update stays a 3D AP after scheduler canonicalization.
    HB = 68
    state = {}
    for s in range(2):
        st_t = stpool.tile([128, 2 * HB], f32, tag=f"state{s}")
        nc.vector.memset(st_t, 0.0)
        state[s] = st_t[64:128, :]

    def two_block(ap2):
        return bass.AP(tensor=ap2.tensor, offset=ap2.offset,
                       ap=[ap2.ap[0], [HB, 2], [1, 64]])

    tab3 = tab.rearrange("c (n d) -> c n d", d=1)

    gbufs = {0: {}, 1: {}}

    def issue_gather(s, c):
        g = gpool.tile([128, TC * B], f32, tag="gbuf")
        g3 = g.rearrange("c (n d) -> c n d", d=1)
        nc.gpsimd.ap_gather(
            g3, tab3, idx[:, (s * T + c * TC) * B // 16:(s * T + (c + 1) * TC) * B // 16],
            channels=128, num_elems=VOCAB, d=1, num_idxs=TC * B,
        )
        gbufs[s][c] = g

    # prime first chunks of both streams
    issue_gather(0, 0)
    issue_gather(1, 0)

    dnames = ("f", "b")
    for n in range(T):
        c = n // TC
        if n % TC == 0 and c + 1 < NCH:
            issue_gather(0, c + 1)
            issue_gather(1, c + 1)
        for s in (0, 1):
            d = dnames[s]
            st = state[s]
            g = gbufs[s][c]
            col = (n % TC) * B
            gx = g[0:65, col:col + B]            # [65, B] x rows + indicator row
            zq = zqpool.tile([128, 128], f32, tag="zq")
            nc.tensor.matmul(zq[:, 0:64], W[f"x_if_{d}"], gx, start=True, stop=False)
            nc.tensor.matmul(zq[:, 0:64], W[f"h_if_{d}"], st[:, HB:HB + 64], start=False, stop=True)
            nc.tensor.matmul(zq[:, 64:128], W[f"x_og_{d}"], gx, start=True, stop=False)
            nc.tensor.matmul(zq[:, 64:128], W[f"h_og_{d}"], st[:, HB:HB + 64], start=False, stop=True)

            S = spool.tile([128, 128], f32, tag="S")
            nc.scalar.activation(S, zq, Sig)
            # i=S[0:64,0:64] f=S[64:128,0:64] gp=S[0:64,64:128] o=S[64:128,64:128]
            # cn = f*c + 2*i*gp - i ; engines chosen to satisfy base-partition
            # pairing (2-input SBUF ops need equal input bases; gpsimd ops
            # additionally keep out at the same base).
            u_t = work.tile([128, 64], f32, tag="u")
            nc.gpsimd.tensor_tensor(u_t[64:128, :], S[64:128, 0:64], st[:, 0:64], op=Op.mult)
            v = work.tile([64, 64], f32, tag="v")
            nc.vector.tensor_tensor(v, S[0:64, 0:64], S[0:64, 64:128], op=Op.mult)
            t3_t = work.tile([128, 64], f32, tag="t3")
            nc.vector.scalar_tensor_tensor(t3_t[64:128, :], v, 2.0, S[0:64, 0:64], op0=Op.mult, op1=Op.subtract)
            new2_t = work.tile([128, 2 * HB], f32, tag="new2")
            new2 = new2_t[64:128, :]
            nc.gpsimd.tensor_tensor(new2[:, 0:64], t3_t[64:128, :], u_t[64:128, :], op=Op.add)
            tc_t = work.tile([128, 64], f32, tag="tcn")
            nc.scalar.activation(tc_t[64:128, :], new2[:, 0:64], Tanh)
            nc.gpsimd.tensor_tensor(new2[:, HB:HB + 64], S[64:128, 64:128], tc_t[64:128, :], op=Op.mult)
            mask2 = g[64:128, col:col + B].bitcast(mybir.dt.uint32).unsqueeze(1).broadcast_to([64, 2, B])
            nc.vector.copy_predicated(two_block(st), mask2, two_block(new2))

    # --- write out: out[b, 0:64] = h_f[:, b]; out[b, 64:128] = h_b[:, b]
    for s in range(2):
        h = state[s][:, HB:HB + 64]
        dst = out[:, s * HID:(s + 1) * HID].transpose((1, 0))  # [hid, b] view of dram
        nc.sync.dma_start(out=dst, in_=h)

    stack.close()


def _compile(T: int, knobs=None):
    import concourse.bacc as bacc
    import concourse.tile as tile
    from concourse import mybir

    key = (T, tuple(sorted((knobs or {}).items())))
    if key in _COMPILED:
        return _COMPILED[key]

    nc = bacc.Bacc("TRN2", num_devices=N_CORES)
    f32 = mybir.dt.float32
    i16 = mybir.dt.int16

    ins = {}
    def din(name, shape, dtype):
        ins[name] = nc.dram_tensor(name, shape, dtype, kind="ExternalInput").ap()

    din("tab", [128, VOCAB], f32)
    din("idx16", [128, 2 * T * B // 16], i16)
    for d in ("f", "b"):
        din(f"lx_if_{d}", [65, 128], f32)
        din(f"lx_og_{d}", [65, 128], f32)
        din(f"lh_if_{d}", [64, 128], f32)
        din(f"lh_og_{d}", [64, 128], f32)
    out = nc.dram_tensor("out", [B, 2 * HID], f32, kind="ExternalOutput").ap()

    with tile.TileContext(nc) as tc:
        _build_body(tc, {"out": out}, ins, T=T, knobs=knobs)
    nc.compile()

    _COMPILED[key] = (nc, list(ins.keys()))
    return _COMPILED[key]


def kernel(tokens, emb_table, Wx_f, Wh_f, b_f, Wx_b, Wh_b, b_b):
    from concourse import bass_utils

    tokens = np.asarray(tokens)
    T = tokens.shape[1]
    nc, in_names = _compile(T)

    shared = _host_prep_shared(
        np.asarray(emb_table), np.asarray(Wx_f), np.asarray(Wh_f), np.asarray(b_f),
        np.asarray(Wx_b), np.asarray(Wh_b), np.asarray(b_b))

    in_maps = []
    for c in range(N_CORES):
        tok_c = tokens[c * B:(c + 1) * B]
        m = dict(shared)
        m["idx16"] = _host_prep_core(tok_c, T)
        in_maps.append(m)

    res = bass_utils.run_bass_kernel_spmd(nc, in_maps, core_ids=list(range(N_CORES)))
    global _LAST_RESULTS, _LAST_EXEC_NS
    _LAST_RESULTS = res
    _LAST_EXEC_NS = getattr(res, "exec_time_ns", None)
    outs = [res.results[c]["out"] for c in range(N_CORES)]
    return np.concatenate(outs, axis=0).astype(np.float32)



# revision 11
# speedup vs baseline: 1.2888x; 1.2888x over previous
"""Bidirectional masked LSTM encoder (B=512, T=1024, EMB=HID=64) on 8 TRN2 cores.

Data-parallel over batch (64 samples/core); fwd+bwd run as two interleaved
per-direction instruction streams.

Key structure (v2):
- x-projections batched per 8-step group as float32r matmuls (N=512 ->
  1 cycle/row) accumulating into PSUM; per-step recurrent h-matmuls are f16
  (1 cycle/row, cheap LDWEIGHTS) and accumulate on top (start=False).
- Gate bias b enters via a K=1 const matmul (ones moving row) per group.
- Keras mask_zero handled by forcing gates on masked steps: gather table
  row 64 holds (1-ind) (ind = token!=0); x-weight row 64 = target - Wx^T
  emb0 - b with targets (i=-30, f=+30, g=0, o=0). Masked steps then give
  f=1, i*g~=0 exactly, so c carries with NO predicated copy. h is carried
  implicitly by carrying o (one small copy_predicated on the o quadrant,
  mask rows 64:128 of the gather output = (1-ind) replicas) and recomputing
  h = o_sel * tanh(c).
- Cell math per step/dir: sigma -> v=i*s_g (DVE), t3=2v-i (DVE),
  u=f*c (GPSIMD), c=t3+u in-place (DVE), tc=tanh(c)->f16 (ACT),
  cp_o (GPSIMD), h=o*tc->f16 (DVE).
"""

import numpy as np

VOCAB = 1000
EMB = 64
HID = 64
B_FULL = 512
T_FULL = 1024
N_CORES = 8
B = B_FULL // N_CORES  # 64 per core
BIG = 30.0

_COMPILED = {}


# ----------------------------------------------------------------------------
# Host-side input packing
# ----------------------------------------------------------------------------

def _pack_wrapped_idx(jw: np.ndarray) -> np.ndarray:
    """ap_gather wrapped layout: index j at partition j%16, slot j//16,
    replicated into each of the eight 16-partition blocks."""
    n = jw.shape[0]
    assert n % 16 == 0
    w = jw.reshape(n // 16, 16).T.astype(np.int16)
    return np.tile(w, (8, 1))


def _host_prep_shared(emb_table, Wx_f, Wh_f, b_f, Wx_b, Wh_b, b_b):
    f32, f16 = np.float32, np.float16
    emb0 = emb_table[0].astype(np.float64)

    def packs(Wx, Wh, b, pfx):
        Wx = Wx.astype(np.float64)
        Wh = Wh.astype(np.float64)
        b = b.astype(np.float64)
        # if-pair: cols 0:128 = (i, f); og-pair: cols (2g, o)
        xw_if = Wx[:, 0:128]
        xw_og = np.hstack([2.0 * Wx[:, 128:192], Wx[:, 192:256]])
        b_if = b[0:128]
        b_og = np.concatenate([2.0 * b[128:192], b[192:256]])
        t_if = np.concatenate([-BIG * np.ones(64), BIG * np.ones(64)])
        t_og = np.zeros(128)
        w64_if = t_if - emb0 @ xw_if - b_if
        w64_og = t_og - emb0 @ xw_og - b_og
        return {
            f"lx_if_{pfx}": np.vstack([xw_if, w64_if[None]]).astype(f32),   # [65,128]
            f"lx_og_{pfx}": np.vstack([xw_og, w64_og[None]]).astype(f32),   # [65,128]
            f"lh_if_{pfx}": Wh[:, 0:128].astype(f16),                        # [64,128]
            f"lh_og_{pfx}": np.hstack(
                [2.0 * Wh[:, 128:192], Wh[:, 192:256]]).astype(f16),         # [64,128]
            f"bc_if_{pfx}": b_if[None, :].astype(f16),                       # [1,128]
            f"bc_og_{pfx}": b_og[None, :].astype(f16),                       # [1,128]
        }

    out = {}
    out.update(packs(Wx_f, Wh_f, b_f, "f"))
    out.update(packs(Wx_b, Wh_b, b_b, "b"))

    inv = (np.arange(VOCAB) == 0).astype(np.float32)  # 1-ind: 1 only for token 0
    tabf = np.vstack(
        [emb_table.T.astype(np.float32), np.tile(inv[None, :], (64, 1))])   # [128,1000]
    out["tab"] = tabf
    th = tabf.astype(np.float16).view(np.uint16).astype(np.uint32)
    out["tab16"] = (th | (th << np.uint32(16))).astype(np.uint32)           # [128,1000]
    for k in list(out):
        if k.startswith("lx_"):
            out[k.replace("lx_", "lx16_")] = out[k].astype(np.float16)
    return out


def _host_prep_core(tok_c: np.ndarray, T: int) -> np.ndarray:
    jw_f = tok_c.T.reshape(-1)                # j = t*B + b
    jw_b = tok_c[:, ::-1].T.reshape(-1)
    return np.concatenate([_pack_wrapped_idx(jw_f), _pack_wrapped_idx(jw_b)], axis=1)


# ----------------------------------------------------------------------------
# Device program
# ----------------------------------------------------------------------------

def _build_body(tc, outs, ins, T: int, knobs=None):
    import concourse.bass as bass
    from concourse import mybir
    from contextlib import ExitStack

    f32 = mybir.dt.float32
    f32r = mybir.dt.float32r
    f16 = mybir.dt.float16
    u32 = mybir.dt.uint32
    Sig = mybir.ActivationFunctionType.Sigmoid
    Tanh = mybir.ActivationFunctionType.Tanh
    Op = mybir.AluOpType

    nc = tc.nc
    TC = 64                      # steps per gather chunk
    NCH = T // TC
    GS = 8                       # steps per psum group
    out = outs["out"]

    stack = ExitStack()
    def pool(name, bufs, **kw):
        return stack.enter_context(tc.tile_pool(name=name, bufs=bufs, **kw))

    kn = {"gbuf": 4, "zq": 2, "sg": 4, "work": 3, "h": 3, "no_f32r": 1}
    kn.update(knobs or {})
    consts = pool("consts", 1)
    gpool = pool("gbuf", kn["gbuf"])
    zqpool = {0: pool("zq_f", kn["zq"], space="PSUM"),
              1: pool("zq_b", kn["zq"], space="PSUM")}
    spool = pool("sg", kn["sg"])
    work = pool("work", kn["work"])
    hpool = pool("h", kn["h"])
    stpool = pool("state", 1)

    # --- constants
    xf16 = kn.get("xf16")
    xdt = u32 if xf16 else (f32 if kn.get("no_f32r") else f32r)
    wdt = f16 if xf16 else xdt
    tab = consts.tile([128, VOCAB], xdt)
    nc.sync.dma_start(out=tab, in_=ins["tab16" if xf16 else "tab"])
    idx = consts.tile([128, 2 * T * B // 16], mybir.dt.int16)
    nc.sync.dma_start(out=idx, in_=ins["idx16"])

    W = {}
    for d in ("f", "b"):
        for p_ in ("if", "og"):
            wx = consts.tile([65, 128], wdt, tag=f"lx_{p_}_{d}")
            nc.sync.dma_start(out=wx, in_=ins[("lx16_" if xf16 else "lx_") + f"{p_}_{d}"])
            wh = consts.tile([64, 128], f16, tag=f"lh_{p_}_{d}")
            nc.sync.dma_start(out=wh, in_=ins[f"lh_{p_}_{d}"])
            bc = consts.tile([1, 128], f16, tag=f"bc_{p_}_{d}")
            nc.sync.dma_start(out=bc, in_=ins[f"bc_{p_}_{d}"])
            W[f"x_{p_}_{d}"] = wx
            W[f"h_{p_}_{d}"] = wh
            W[f"b_{p_}_{d}"] = bc

    ones_mv = consts.tile([1, GS * B], f16)
    nc.vector.memset(ones_mv, 1.0)
    zero_h = consts.tile([128, B], f16)          # step-0 dummy h (rows 0:64)
    nc.vector.memset(zero_h, 0.0)
    zero_o = consts.tile([128, B], f32)          # initial o_sel (rows 64:128)
    nc.vector.memset(zero_o, 0.0)

    # --- per-direction persistent c state (rows 64:128)
    cst = {}
    for s in range(2):
        t_ = stpool.tile([128, B], f32, tag=f"c{s}")
        nc.vector.memset(t_, 0.0)
        cst[s] = t_

    tab3 = tab.rearrange("c (n d) -> c n d", d=1)
    gbufs = {0: {}, 1: {}}

    def issue_gather(s, c):
        g = gpool.tile([128, TC * B], xdt, tag="gbuf")
        g3 = g.rearrange("c (n d) -> c n d", d=1)
        nc.gpsimd.ap_gather(
            g3, tab3, idx[:, (s * T + c * TC) * B // 16:(s * T + (c + 1) * TC) * B // 16],
            channels=128, num_elems=VOCAB, d=1, num_idxs=TC * B,
        )
        gbufs[s][c] = g

    issue_gather(0, 0)
    issue_gather(1, 0)

    dnames = ("f", "b")
    zqt = {0: {}, 1: {}}          # live psum group tiles per dir
    o_prev = {0: zero_o[64:128, :], 1: zero_o[64:128, :]}
    h_prev = {0: zero_h, 1: zero_h}
    h_last = {}

    def issue_group(s, gidx):
        """const + x matmuls for steps [gidx*GS, (gidx+1)*GS) of direction s."""
        d = dnames[s]
        g = gbufs[s][(gidx * GS) // TC]
        col = ((gidx * GS) % TC) * B
        zq = zqpool[s].tile([128, 2 * GS * B], f32, tag="zq")
        if xf16:
            gf = g.bitcast(f16)
            gx = bass.AP(tensor=gf.tensor, offset=gf.offset + 2 * col,
                         ap=[[gf.ap[0][0], 65], [2, GS * B]])
        else:
            gx = g[0:65, col:col + GS * B]
        if kn.get("no_const"):
            nc.tensor.matmul(zq[:, 0:GS * B], W[f"x_if_{d}"], gx,
                             start=True, stop=False, skip_group_check=True)
            nc.tensor.matmul(zq[:, GS * B:2 * GS * B], W[f"x_og_{d}"], gx,
                             start=True, stop=False, skip_group_check=True)
        else:
            nc.tensor.matmul(zq[:, 0:GS * B], W[f"b_if_{d}"], ones_mv,
                             start=True, stop=False, skip_group_check=True)
            nc.tensor.matmul(zq[:, 0:GS * B], W[f"x_if_{d}"], gx,
                             start=False, stop=False, skip_group_check=True)
            nc.tensor.matmul(zq[:, GS * B:2 * GS * B], W[f"b_og_{d}"], ones_mv,
                             start=True, stop=False, skip_group_check=True)
            nc.tensor.matmul(zq[:, GS * B:2 * GS * B], W[f"x_og_{d}"], gx,
                             start=False, stop=False, skip_group_check=True)
        zqt[s][gidx] = zq

    issue_group(0, 0)
    issue_group(1, 0)

    NG = T // GS
    for n in range(T):
        c_ = n // TC
        if n % TC == 0 and c_ + 1 < NCH:
            issue_gather(0, c_ + 1)
            issue_gather(1, c_ + 1)
        for s in (0, 1):
            d = dnames[s]
            gidx, k = n // GS, n % GS
            zq = zqt[s][gidx]
            # h-part of step n's gates (h from step n-1; zeros at n=0)
            hmv = h_prev[s][0:64, :]
            nc.tensor.matmul(zq[:, k * B:(k + 1) * B], W[f"h_if_{d}"], hmv,
                             start=False, stop=True, skip_group_check=True)
            nc.tensor.matmul(zq[:, (GS + k) * B:(GS + k + 1) * B], W[f"h_og_{d}"], hmv,
                             start=False, stop=True, skip_group_check=True)
            if k == GS - 1 and gidx + 1 < NG:
                issue_group(s, gidx + 1)

            # gates: sigma over [if | og] blocks of this step
            zv = bass.AP(tensor=zq.tensor, offset=zq.offset + k * B,
                         ap=[zq.ap[0], [GS * B, 2], [1, B]])
            S = spool.tile([128, 2 * B], f32, tag=f"S{s}")
            S3 = S.rearrange("p (n d) -> p n d", d=B)
            nc.scalar.activation(S3, zv, Sig)
            # i=S[0:64,0:B] f=S[64:128,0:B] sg=S[0:64,B:2B] o=S[64:128,B:2B]
            v = work.tile([128, B], f32, tag=f"v{s}")
            nc.vector.tensor_tensor(v[0:64, :], S[0:64, 0:B], S[0:64, B:2 * B], op=Op.mult)
            t3 = work.tile([128, B], f32, tag=f"t3{s}")
            nc.vector.scalar_tensor_tensor(
                t3[64:128, :], v[0:64, :], 2.0, S[0:64, 0:B], op0=Op.mult, op1=Op.subtract)
            u = work.tile([128, B], f32, tag=f"u{s}")
            nc.gpsimd.tensor_tensor(u[64:128, :], S[64:128, 0:B], cst[s][64:128, :], op=Op.mult)
            nc.vector.tensor_tensor(cst[s][64:128, :], t3[64:128, :], u[64:128, :], op=Op.add)
            tcn = work.tile([128, B], f16, tag=f"tc{s}")
            nc.scalar.activation(tcn[64:128, :], cst[s][64:128, :], Tanh)
            # o carry: overwrite o with previous o_sel where masked (1-ind != 0)
            g = gbufs[s][c_]
            gm = g if xf16 else g.bitcast(u32)
            mask = gm[64:128, (n % TC) * B:(n % TC) * B + B]
            nc.vector.copy_predicated(S[64:128, B:2 * B], mask, o_prev[s])
            o_prev[s] = S[64:128, B:2 * B]
            h = hpool.tile([128, B], f16, tag=f"h{s}")
            nc.vector.tensor_tensor(h[0:64, :], S[64:128, B:2 * B], tcn[64:128, :], op=Op.mult)
            h_prev[s] = h
            h_last[s] = h

    # --- write out: out[b, s*64:(s+1)*64] = h_s[:, b]
    for s in range(2):
        hf = work.tile([128, B], f32, tag=f"hf{s}")
        nc.vector.tensor_copy(hf[0:64, :], h_last[s][0:64, :])
        dst = out[:, s * HID:(s + 1) * HID].transpose((1, 0))
        nc.sync.dma_start(out=dst, in_=hf[0:64, :])

    stack.close()


def _compile(T: int, knobs=None):
    import concourse.bacc as bacc
    import concourse.tile as tile
    from concourse import mybir

    key = (T, tuple(sorted((knobs or {}).items())))
    if key in _COMPILED:
        return _COMPILED[key]

    nc = bacc.Bacc("TRN2", num_devices=N_CORES)
    f32 = mybir.dt.float32
    f32r = mybir.dt.float32r
    f16 = mybir.dt.float16
    i16 = mybir.dt.int16

    ins = {}
    def din(name, shape, dtype):
        ins[name] = nc.dram_tensor(name, shape, dtype, kind="ExternalInput").ap()

    kn_ = dict(knobs or {})
    if kn_.get("xf16"):
        din("tab16", [128, VOCAB], mybir.dt.uint32)
    else:
        din("tab", [128, VOCAB], f32 if kn_.get("no_f32r", 1) else f32r)
    din("idx16", [128, 2 * T * B // 16], i16)
    for d in ("f", "b"):
        if kn_.get("xf16"):
            din(f"lx16_if_{d}", [65, 128], f16)
            din(f"lx16_og_{d}", [65, 128], f16)
        else:
            din(f"lx_if_{d}", [65, 128], f32 if kn_.get("no_f32r", 1) else f32r)
            din(f"lx_og_{d}", [65, 128], f32 if kn_.get("no_f32r", 1) else f32r)
        din(f"lh_if_{d}", [64, 128], f16)
        din(f"lh_og_{d}", [64, 128], f16)
        din(f"bc_if_{d}", [1, 128], f16)
        din(f"bc_og_{d}", [1, 128], f16)
    out = nc.dram_tensor("out", [B, 2 * HID], f32, kind="ExternalOutput").ap()

    with tile.TileContext(nc) as tc:
        _build_body(tc, {"out": out}, ins, T=T, knobs=knobs)
    nc.compile()

    _COMPILED[key] = (nc, list(ins.keys()))
    return _COMPILED[key]


def kernel(tokens, emb_table, Wx_f, Wh_f, b_f, Wx_b, Wh_b, b_b):
    from concourse import bass_utils

    tokens = np.asarray(tokens)
    T = tokens.shape[1]
    nc, in_names = _compile(T)

    shared = _host_prep_shared(
        np.asarray(emb_table), np.asarray(Wx_f), np.asarray(Wh_f), np.asarray(b_f),
        np.asarray(Wx_b), np.asarray(Wh_b), np.asarray(b_b))

    in_maps = []
    for c in range(N_CORES):
        tok_c = tokens[c * B:(c + 1) * B]
        m = dict(shared)
        m["idx16"] = _host_prep_core(tok_c, T)
        in_maps.append(m)

    res = bass_utils.run_bass_kernel_spmd(nc, in_maps, core_ids=list(range(N_CORES)))
    global _LAST_RESULTS, _LAST_EXEC_NS
    _LAST_RESULTS = res
    _LAST_EXEC_NS = getattr(res, "exec_time_ns", None)
    outs = [res.results[c]["out"] for c in range(N_CORES)]
    return np.concatenate(outs, axis=0).astype(np.float32)


# revision 13
# speedup vs baseline: 1.9855x; 1.5406x over previous
"""Bidirectional masked LSTM encoder (B=512, T=1024, EMB=HID=64) on 8 TRN2 cores.

Data-parallel over batch (64 samples/core); fwd+bwd run as two interleaved
per-direction instruction streams.

Key structure (v2):
- x-projections batched per 8-step group as float32r matmuls (N=512 ->
  1 cycle/row) accumulating into PSUM; per-step recurrent h-matmuls are f16
  (1 cycle/row, cheap LDWEIGHTS) and accumulate on top (start=False).
- Gate bias b enters via a K=1 const matmul (ones moving row) per group.
- Keras mask_zero handled by forcing gates on masked steps: gather table
  row 64 holds (1-ind) (ind = token!=0); x-weight row 64 = target - Wx^T
  emb0 - b with targets (i=-30, f=+30, g=0, o=0). Masked steps then give
  f=1, i*g~=0 exactly, so c carries with NO predicated copy. h is carried
  implicitly by carrying o (one small copy_predicated on the o quadrant,
  mask rows 64:128 of the gather output = (1-ind) replicas) and recomputing
  h = o_sel * tanh(c).
- Cell math per step/dir: sigma -> v=i*s_g (DVE), t3=2v-i (DVE),
  u=f*c (GPSIMD), c=t3+u in-place (DVE), tc=tanh(c)->f16 (ACT),
  cp_o (GPSIMD), h=o*tc->f16 (DVE).
"""

import numpy as np

VOCAB = 1000
EMB = 64
HID = 64
B_FULL = 512
T_FULL = 1024
N_CORES = 8
B = B_FULL // N_CORES  # 64 per core
BIG = 30.0

_COMPILED = {}


# ----------------------------------------------------------------------------
# Host-side input packing
# ----------------------------------------------------------------------------

def _pack_wrapped_idx(jw: np.ndarray) -> np.ndarray:
    """ap_gather wrapped layout: index j at partition j%16, slot j//16,
    replicated into each of the eight 16-partition blocks."""
    n = jw.shape[0]
    assert n % 16 == 0
    w = jw.reshape(n // 16, 16).T.astype(np.int16)
    return np.tile(w, (8, 1))


def _host_prep_shared(emb_table, Wx_f, Wh_f, b_f, Wx_b, Wh_b, b_b):
    f32, f16 = np.float32, np.float16
    emb0 = emb_table[0].astype(np.float64)

    def packs(Wx, Wh, b, pfx):
        Wx = Wx.astype(np.float64)
        Wh = Wh.astype(np.float64)
        b = b.astype(np.float64)
        # if-pair: cols 0:128 = (i, f); og-pair: cols (2g, o)
        xw_if = Wx[:, 0:128]
        xw_og = np.hstack([2.0 * Wx[:, 128:192], Wx[:, 192:256]])
        b_if = b[0:128]
        b_og = np.concatenate([2.0 * b[128:192], b[192:256]])
        t_if = np.concatenate([-BIG * np.ones(64), BIG * np.ones(64)])
        t_og = np.zeros(128)
        w64_if = t_if - emb0 @ xw_if - b_if
        w64_og = t_og - emb0 @ xw_og - b_og
        return {
            f"lx_if_{pfx}": np.vstack([xw_if, w64_if[None]]).astype(f32),   # [65,128]
            f"lx_og_{pfx}": np.vstack([xw_og, w64_og[None]]).astype(f32),   # [65,128]
            f"lh_if_{pfx}": Wh[:, 0:128].astype(f16),                        # [64,128]
            f"lh_og_{pfx}": np.hstack(
                [2.0 * Wh[:, 128:192], Wh[:, 192:256]]).astype(f16),         # [64,128]
            f"bc_if_{pfx}": b_if[None, :].astype(f16),                       # [1,128]
            f"bc_og_{pfx}": b_og[None, :].astype(f16),                       # [1,128]
        }

    out = {}
    out.update(packs(Wx_f, Wh_f, b_f, "f"))
    out.update(packs(Wx_b, Wh_b, b_b, "b"))

    inv = (np.arange(VOCAB) == 0).astype(np.float32)  # 1-ind: 1 only for token 0
    tabf = np.vstack(
        [emb_table.T.astype(np.float32), np.tile(inv[None, :], (64, 1))])   # [128,1000]
    out["tab"] = tabf
    th = tabf.astype(np.float16).view(np.uint16).astype(np.uint32)
    out["tab16"] = (th | (th << np.uint32(16))).astype(np.uint32)           # [128,1000]
    for k in list(out):
        if k.startswith("lx_"):
            out[k.replace("lx_", "lx16_")] = out[k].astype(np.float16)
    return out


def _host_prep_core(tok_c: np.ndarray, T: int) -> np.ndarray:
    jw_f = tok_c.T.reshape(-1)                # j = t*B + b
    jw_b = tok_c[:, ::-1].T.reshape(-1)
    return np.concatenate([_pack_wrapped_idx(jw_f), _pack_wrapped_idx(jw_b)], axis=1)


# ----------------------------------------------------------------------------
# Device program
# ----------------------------------------------------------------------------

def _build_body(tc, outs, ins, T: int, knobs=None):
    import concourse.bass as bass
    from concourse import mybir
    from contextlib import ExitStack

    f32 = mybir.dt.float32
    f32r = mybir.dt.float32r
    f16 = mybir.dt.float16
    u32 = mybir.dt.uint32
    Sig = mybir.ActivationFunctionType.Sigmoid
    Tanh = mybir.ActivationFunctionType.Tanh
    Op = mybir.AluOpType

    nc = tc.nc
    TC = 64                      # steps per gather chunk
    NCH = T // TC
    GS = 8                       # steps per psum group
    out = outs["out"]

    stack = ExitStack()
    def pool(name, bufs, **kw):
        return stack.enter_context(tc.tile_pool(name=name, bufs=bufs, **kw))

    kn = {"gbuf": 4, "zq": 2, "sg": 4, "work": 3, "h": 3, "no_f32r": 1}
    kn.update(knobs or {})
    consts = pool("consts", 1)
    gpool = pool("gbuf", kn["gbuf"])
    zqpool = {0: pool("zq_f", kn["zq"], space="PSUM"),
              1: pool("zq_b", kn["zq"], space="PSUM")}
    spool = pool("sg", kn["sg"])
    work = pool("work", kn["work"])
    hpool = pool("h", kn["h"])
    stpool = pool("state", 1)

    # --- constants
    xf16 = kn.get("xf16")
    xdt = u32 if xf16 else (f32 if kn.get("no_f32r") else f32r)
    wdt = f16 if xf16 else xdt
    tab = consts.tile([128, VOCAB], xdt)
    nc.sync.dma_start(out=tab, in_=ins["tab16" if xf16 else "tab"])
    idx = consts.tile([128, 2 * T * B // 16], mybir.dt.int16)
    nc.sync.dma_start(out=idx, in_=ins["idx16"])

    W = {}
    for d in ("f", "b"):
        for p_ in ("if", "og"):
            wx = consts.tile([65, 128], wdt, tag=f"lx_{p_}_{d}")
            nc.sync.dma_start(out=wx, in_=ins[("lx16_" if xf16 else "lx_") + f"{p_}_{d}"])
            wh = consts.tile([64, 128], f16, tag=f"lh_{p_}_{d}")
            nc.sync.dma_start(out=wh, in_=ins[f"lh_{p_}_{d}"])
            bc = consts.tile([1, 128], f16, tag=f"bc_{p_}_{d}")
            nc.sync.dma_start(out=bc, in_=ins[f"bc_{p_}_{d}"])
            W[f"x_{p_}_{d}"] = wx
            W[f"h_{p_}_{d}"] = wh
            W[f"b_{p_}_{d}"] = bc

    ones_mv = consts.tile([1, GS * B], f16)
    nc.vector.memset(ones_mv, 1.0)
    zero_h = consts.tile([128, B], f16)          # step-0 dummy h (rows 0:64)
    nc.vector.memset(zero_h, 0.0)
    zero_o = consts.tile([128, B], f32)          # initial o_sel (rows 64:128)
    nc.vector.memset(zero_o, 0.0)

    # --- per-direction persistent c state (rows 64:128)
    cst = {}
    for s in range(2):
        t_ = stpool.tile([128, B], f32, tag=f"c{s}")
        nc.vector.memset(t_, 0.0)
        cst[s] = t_

    tab3 = tab.rearrange("c (n d) -> c n d", d=1)
    gbufs = {0: {}, 1: {}}

    def issue_gather(s, c):
        g = gpool.tile([128, TC * B], xdt, tag="gbuf")
        g3 = g.rearrange("c (n d) -> c n d", d=1)
        nc.gpsimd.ap_gather(
            g3, tab3, idx[:, (s * T + c * TC) * B // 16:(s * T + (c + 1) * TC) * B // 16],
            channels=128, num_elems=VOCAB, d=1, num_idxs=TC * B,
        )
        gbufs[s][c] = g

    issue_gather(0, 0)
    issue_gather(1, 0)

    dnames = ("f", "b")
    zqt = {0: {}, 1: {}}          # live psum group tiles per dir
    o_prev = {0: zero_o[64:128, :], 1: zero_o[64:128, :]}
    h_prev = {0: zero_h, 1: zero_h}
    h_last = {}

    def issue_group(s, gidx):
        """const + x matmuls for steps [gidx*GS, (gidx+1)*GS) of direction s."""
        d = dnames[s]
        g = gbufs[s][(gidx * GS) // TC]
        col = ((gidx * GS) % TC) * B
        zq = zqpool[s].tile([128, 2 * GS * B], f32, tag="zq")
        if xf16:
            gf = g.bitcast(f16)
            gx = bass.AP(tensor=gf.tensor, offset=gf.offset + 2 * col,
                         ap=[[gf.ap[0][0], 65], [2, GS * B]])
        else:
            gx = g[0:65, col:col + GS * B]
        if kn.get("no_const"):
            nc.tensor.matmul(zq[:, 0:GS * B], W[f"x_if_{d}"], gx,
                             start=True, stop=False, skip_group_check=True)
            nc.tensor.matmul(zq[:, GS * B:2 * GS * B], W[f"x_og_{d}"], gx,
                             start=True, stop=False, skip_group_check=True)
        else:
            nc.tensor.matmul(zq[:, 0:GS * B], W[f"b_if_{d}"], ones_mv,
                             start=True, stop=False, skip_group_check=True)
            nc.tensor.matmul(zq[:, 0:GS * B], W[f"x_if_{d}"], gx,
                             start=False, stop=False, skip_group_check=True)
            nc.tensor.matmul(zq[:, GS * B:2 * GS * B], W[f"b_og_{d}"], ones_mv,
                             start=True, stop=False, skip_group_check=True)
            nc.tensor.matmul(zq[:, GS * B:2 * GS * B], W[f"x_og_{d}"], gx,
                             start=False, stop=False, skip_group_check=True)
        zqt[s][gidx] = zq

    issue_group(0, 0)
    issue_group(1, 0)

    NG = T // GS
    for n in range(T):
        c_ = n // TC
        if n % TC == 0 and c_ + 1 < NCH:
            issue_gather(0, c_ + 1)
            issue_gather(1, c_ + 1)
        for s in (0, 1):
            d = dnames[s]
            gidx, k = n // GS, n % GS
            zq = zqt[s][gidx]
            # h-part of step n's gates (h from step n-1; zeros at n=0)
            hmv = h_prev[s][0:64, :]
            nc.tensor.matmul(zq[:, k * B:(k + 1) * B], W[f"h_if_{d}"], hmv,
                             start=False, stop=True, skip_group_check=True)
            nc.tensor.matmul(zq[:, (GS + k) * B:(GS + k + 1) * B], W[f"h_og_{d}"], hmv,
                             start=False, stop=True, skip_group_check=True)
            if k == GS - 1 and gidx + 1 < NG:
                issue_group(s, gidx + 1)

            # gates: sigma over [if | og] blocks of this step
            zv = bass.AP(tensor=zq.tensor, offset=zq.offset + k * B,
                         ap=[zq.ap[0], [GS * B, 2], [1, B]])
            S = spool.tile([128, 2 * B], f32, tag=f"S{s}")
            S3 = S.rearrange("p (n d) -> p n d", d=B)
            nc.scalar.activation(S3, zv, Sig)
            # i=S[0:64,0:B] f=S[64:128,0:B] sg=S[0:64,B:2B] o=S[64:128,B:2B]
            v = work.tile([128, B], f32, tag=f"v{s}")
            nc.vector.tensor_tensor(v[0:64, :], S[0:64, 0:B], S[0:64, B:2 * B], op=Op.mult)
            t3 = work.tile([128, B], f32, tag=f"t3{s}")
            nc.vector.scalar_tensor_tensor(
                t3[64:128, :], v[0:64, :], 2.0, S[0:64, 0:B], op0=Op.mult, op1=Op.subtract)
            u = work.tile([128, B], f32, tag=f"u{s}")
            nc.vector.tensor_tensor(u[64:128, :], S[64:128, 0:B], cst[s][64:128, :], op=Op.mult)
            nc.vector.tensor_tensor(cst[s][64:128, :], t3[64:128, :], u[64:128, :], op=Op.add)
            tcn = work.tile([128, B], f16, tag=f"tc{s}")
            nc.scalar.activation(tcn[64:128, :], cst[s][64:128, :], Tanh)
            # o carry: overwrite o with previous o_sel where masked (1-ind != 0)
            g = gbufs[s][c_]
            gm = g if xf16 else g.bitcast(u32)
            mask = gm[64:128, (n % TC) * B:(n % TC) * B + B]
            nc.vector.copy_predicated(S[64:128, B:2 * B], mask, o_prev[s])
            o_prev[s] = S[64:128, B:2 * B]
            h = hpool.tile([128, B], f16, tag=f"h{s}")
            nc.vector.tensor_tensor(h[0:64, :], S[64:128, B:2 * B], tcn[64:128, :], op=Op.mult)
            h_prev[s] = h
            h_last[s] = h

    # --- write out: out[b, s*64:(s+1)*64] = h_s[:, b]
    for s in range(2):
        hf = work.tile([128, B], f32, tag=f"hf{s}")
        nc.vector.tensor_copy(hf[0:64, :], h_last[s][0:64, :])
        dst = out[:, s * HID:(s + 1) * HID].transpose((1, 0))
        nc.sync.dma_start(out=dst, in_=hf[0:64, :])

    stack.close()


def _compile(T: int, knobs=None):
    import concourse.bacc as bacc
    import concourse.tile as tile
    from concourse import mybir

    key = (T, tuple(sorted((knobs or {}).items())))
    if key in _COMPILED:
        return _COMPILED[key]

    nc = bacc.Bacc("TRN2", num_devices=N_CORES)
    f32 = mybir.dt.float32
    f32r = mybir.dt.float32r
    f16 = mybir.dt.float16
    i16 = mybir.dt.int16

    ins = {}
    def din(name, shape, dtype):
        ins[name] = nc.dram_tensor(name, shape, dtype, kind="ExternalInput").ap()

    kn_ = dict(knobs or {})
    if kn_.get("xf16"):
        din("tab16", [128, VOCAB], mybir.dt.uint32)
    else:
        din("tab", [128, VOCAB], f32 if kn_.get("no_f32r", 1) else f32r)
    din("idx16", [128, 2 * T * B // 16], i16)
    for d in ("f", "b"):
        if kn_.get("xf16"):
            din(f"lx16_if_{d}", [65, 128], f16)
            din(f"lx16_og_{d}", [65, 128], f16)
        else:
            din(f"lx_if_{d}", [65, 128], f32 if kn_.get("no_f32r", 1) else f32r)
            din(f"lx_og_{d}", [65, 128], f32 if kn_.get("no_f32r", 1) else f32r)
        din(f"lh_if_{d}", [64, 128], f16)
        din(f"lh_og_{d}", [64, 128], f16)
        din(f"bc_if_{d}", [1, 128], f16)
        din(f"bc_og_{d}", [1, 128], f16)
    out = nc.dram_tensor("out", [B, 2 * HID], f32, kind="ExternalOutput").ap()

    with tile.TileContext(nc) as tc:
        _build_body(tc, {"out": out}, ins, T=T, knobs=knobs)
    nc.compile()

    _COMPILED[key] = (nc, list(ins.keys()))
    return _COMPILED[key]


def kernel(tokens, emb_table, Wx_f, Wh_f, b_f, Wx_b, Wh_b, b_b):
    from concourse import bass_utils

    tokens = np.asarray(tokens)
    T = tokens.shape[1]
    nc, in_names = _compile(T)

    shared = _host_prep_shared(
        np.asarray(emb_table), np.asarray(Wx_f), np.asarray(Wh_f), np.asarray(b_f),
        np.asarray(Wx_b), np.asarray(Wh_b), np.asarray(b_b))

    in_maps = []
    for c in range(N_CORES):
        tok_c = tokens[c * B:(c + 1) * B]
        m = {k: v for k, v in shared.items() if k in in_names}
        m["idx16"] = _host_prep_core(tok_c, T)
        in_maps.append(m)

    res = bass_utils.run_bass_kernel_spmd(nc, in_maps, core_ids=list(range(N_CORES)))
    global _LAST_RESULTS, _LAST_EXEC_NS
    _LAST_RESULTS = res
    _LAST_EXEC_NS = getattr(res, "exec_time_ns", None)
    outs = [res.results[c]["out"] for c in range(N_CORES)]
    return np.concatenate(outs, axis=0).astype(np.float32)


# revision 21
# speedup vs baseline: 2.0386x; 1.0268x over previous
"""Bidirectional masked LSTM encoder (B=512, T=1024, EMB=HID=64) on 8 TRN2 cores.

Data-parallel over batch (64 samples/core); fwd+bwd run as two interleaved
per-direction instruction streams.

Key structure (v2):
- x-projections batched per 8-step group as float32r matmuls (N=512 ->
  1 cycle/row) accumulating into PSUM; per-step recurrent h-matmuls are f16
  (1 cycle/row, cheap LDWEIGHTS) and accumulate on top (start=False).
- Gate bias b enters via a K=1 const matmul (ones moving row) per group.
- Keras mask_zero handled by forcing gates on masked steps: gather table
  row 64 holds (1-ind) (ind = token!=0); x-weight row 64 = target - Wx^T
  emb0 - b with targets (i=-30, f=+30, g=0, o=0). Masked steps then give
  f=1, i*g~=0 exactly, so c carries with NO predicated copy. h is carried
  implicitly by carrying o (one small copy_predicated on the o quadrant,
  mask rows 64:128 of the gather output = (1-ind) replicas) and recomputing
  h = o_sel * tanh(c).
- Cell math per step/dir: sigma -> v=i*s_g (DVE), t3=2v-i (DVE),
  u=f*c (GPSIMD), c=t3+u in-place (DVE), tc=tanh(c)->f16 (ACT),
  cp_o (GPSIMD), h=o*tc->f16 (DVE).
"""

import numpy as np

VOCAB = 1000
EMB = 64
HID = 64
B_FULL = 512
T_FULL = 1024
N_CORES = 8
B = B_FULL // N_CORES  # 64 per core
BIG = 30.0

_COMPILED = {}
DEFAULT_KNOBS = {"gbuf": 4, "zq": 2, "sg": 4, "work": 3, "h": 3, "no_f32r": 1, "xf16": 1}


# ----------------------------------------------------------------------------
# Host-side input packing
# ----------------------------------------------------------------------------

def _pack_wrapped_idx(jw: np.ndarray) -> np.ndarray:
    """ap_gather wrapped layout: index j at partition j%16, slot j//16,
    replicated into each of the eight 16-partition blocks."""
    n = jw.shape[0]
    assert n % 16 == 0
    w = jw.reshape(n // 16, 16).T.astype(np.int16)
    return np.tile(w, (8, 1))


def _host_prep_shared(emb_table, Wx_f, Wh_f, b_f, Wx_b, Wh_b, b_b):
    f32, f16 = np.float32, np.float16
    emb0 = emb_table[0].astype(np.float64)

    def packs(Wx, Wh, b, pfx):
        Wx = Wx.astype(np.float64)
        Wh = Wh.astype(np.float64)
        b = b.astype(np.float64)
        # if-pair: cols 0:128 = (i, f); og-pair: cols (2g, o)
        xw_if = Wx[:, 0:128]
        xw_og = np.hstack([2.0 * Wx[:, 128:192], Wx[:, 192:256]])
        b_if = b[0:128]
        b_og = np.concatenate([2.0 * b[128:192], b[192:256]])
        t_if = np.concatenate([-BIG * np.ones(64), BIG * np.ones(64)])
        t_og = np.zeros(128)
        w64_if = t_if - emb0 @ xw_if - b_if
        w64_og = t_og - emb0 @ xw_og - b_og
        return {
            f"lx_if_{pfx}": np.vstack([xw_if, w64_if[None]]).astype(f32),   # [65,128]
            f"lx_og_{pfx}": np.vstack([xw_og, w64_og[None]]).astype(f32),   # [65,128]
            f"lh_if_{pfx}": Wh[:, 0:128].astype(f16),                        # [64,128]
            f"lh_og_{pfx}": np.hstack(
                [2.0 * Wh[:, 128:192], Wh[:, 192:256]]).astype(f16),         # [64,128]
            f"bc_if_{pfx}": b_if[None, :].astype(f16),                       # [1,128]
            f"bc_og_{pfx}": b_og[None, :].astype(f16),                       # [1,128]
        }

    out = {}
    out.update(packs(Wx_f, Wh_f, b_f, "f"))
    out.update(packs(Wx_b, Wh_b, b_b, "b"))

    inv = (np.arange(VOCAB) == 0).astype(np.float32)  # 1-ind: 1 only for token 0
    tabf = np.vstack(
        [emb_table.T.astype(np.float32), np.tile(inv[None, :], (64, 1))])   # [128,1000]
    out["tab"] = tabf
    th = tabf.astype(np.float16).view(np.uint16).astype(np.uint32)
    out["tab16"] = (th | (th << np.uint32(16))).astype(np.uint32)           # [128,1000]
    for k in list(out):
        if k.startswith("lx_"):
            out[k.replace("lx_", "lx16_")] = out[k].astype(np.float16)
    return out


def _host_prep_core(tok_c: np.ndarray, T: int) -> np.ndarray:
    jw_f = tok_c.T.reshape(-1)                # j = t*B + b
    jw_b = tok_c[:, ::-1].T.reshape(-1)
    return np.concatenate([_pack_wrapped_idx(jw_f), _pack_wrapped_idx(jw_b)], axis=1)


# ----------------------------------------------------------------------------
# Device program
# ----------------------------------------------------------------------------

def _build_body(tc, outs, ins, T: int, knobs=None):
    import concourse.bass as bass
    from concourse import mybir
    from contextlib import ExitStack

    f32 = mybir.dt.float32
    f32r = mybir.dt.float32r
    f16 = mybir.dt.float16
    u32 = mybir.dt.uint32
    Sig = mybir.ActivationFunctionType.Sigmoid
    Tanh = mybir.ActivationFunctionType.Tanh
    Op = mybir.AluOpType

    nc = tc.nc
    TC = 64                      # steps per gather chunk
    NCH = T // TC
    GS = 8                       # steps per psum group
    out = outs["out"]

    stack = ExitStack()
    def pool(name, bufs, **kw):
        return stack.enter_context(tc.tile_pool(name=name, bufs=bufs, **kw))

    kn = dict(DEFAULT_KNOBS)
    kn.update(knobs or {})
    consts = pool("consts", 1)
    gpool = pool("gbuf", kn["gbuf"])
    zqpool = {0: pool("zq_f", kn["zq"], space="PSUM"),
              1: pool("zq_b", kn["zq"], space="PSUM")}
    spool = pool("sg", kn["sg"])
    work = pool("work", kn["work"])
    hpool = pool("h", kn["h"])
    stpool = pool("state", 1)

    # --- constants
    xf16 = kn.get("xf16")
    xdt = u32 if xf16 else (f32 if kn.get("no_f32r") else f32r)
    wdt = f16 if xf16 else xdt
    tab = consts.tile([128, VOCAB], xdt)
    nc.sync.dma_start(out=tab, in_=ins["tab16" if xf16 else "tab"])
    idx = consts.tile([128, 2 * T * B // 16], mybir.dt.int16)
    nc.sync.dma_start(out=idx, in_=ins["idx16"])

    W = {}
    for d in ("f", "b"):
        for p_ in ("if", "og"):
            wx = consts.tile([65, 128], wdt, tag=f"lx_{p_}_{d}")
            nc.sync.dma_start(out=wx, in_=ins[("lx16_" if xf16 else "lx_") + f"{p_}_{d}"])
            wh = consts.tile([64, 128], f16, tag=f"lh_{p_}_{d}")
            nc.sync.dma_start(out=wh, in_=ins[f"lh_{p_}_{d}"])
            bc = consts.tile([1, 128], f16, tag=f"bc_{p_}_{d}")
            nc.sync.dma_start(out=bc, in_=ins[f"bc_{p_}_{d}"])
            W[f"x_{p_}_{d}"] = wx
            W[f"h_{p_}_{d}"] = wh
            W[f"b_{p_}_{d}"] = bc

    ones_mv = consts.tile([1, GS * B], f16)
    nc.vector.memset(ones_mv, 1.0)
    zero_h = consts.tile([128, B], f16)          # step-0 dummy h (rows 0:64)
    nc.vector.memset(zero_h, 0.0)
    zero_o = consts.tile([128, B], f32)          # initial o_sel (rows 64:128)
    nc.vector.memset(zero_o, 0.0)

    # --- per-direction persistent c state (rows 64:128)
    cst = {}
    for s in range(2):
        t_ = stpool.tile([128, B], f32, tag=f"c{s}")
        nc.vector.memset(t_, 0.0)
        cst[s] = t_

    tab3 = tab.rearrange("c (n d) -> c n d", d=1)
    gbufs = {0: {}, 1: {}}

    def do_gather(s, c, g):
        g3 = g.rearrange("c (n d) -> c n d", d=1)
        nc.gpsimd.ap_gather(
            g3, tab3, idx[:, (s * T + c * TC) * B // 16:(s * T + (c + 1) * TC) * B // 16],
            channels=128, num_elems=VOCAB, d=1, num_idxs=TC * B,
        )

    stage = kn.get("stage")
    if stage:
        # Gather every chunk up-front (gpsimd runs ONLY gathers: one library
        # load), stage to DRAM, and stream chunks back per-chunk via DMA.
        fpool = pool("fbuf", kn["gbuf"])
        dpool = pool("xsd", 2 * NCH, space="DRAM")
        xs = {}
        for c in range(NCH):
            for s in (0, 1):
                g = gpool.tile([128, TC * B], xdt, tag="gbuf")
                do_gather(s, c, g)
                xt = dpool.tile([128, TC * B], xdt, tag="xs")
                nc.sync.dma_start(out=xt, in_=g)
                xs[(s, c)] = xt

        def issue_fetch(s, c):
            g2 = fpool.tile([128, TC * B], xdt, tag="fb")
            nc.sync.dma_start(out=g2, in_=xs[(s, c)])
            gbufs[s][c] = g2
    else:
        def issue_fetch(s, c):
            g = gpool.tile([128, TC * B], xdt, tag="gbuf")
            do_gather(s, c, g)
            gbufs[s][c] = g

    issue_fetch(0, 0)
    issue_fetch(1, 0)

    dnames = ("f", "b")
    zqt = {0: {}, 1: {}}          # live psum group tiles per dir
    o_prev = {0: zero_o[64:128, :], 1: zero_o[64:128, :]}
    h_prev = {0: zero_h, 1: zero_h}
    h_last = {}

    def issue_group(s, gidx):
        """const + x matmuls for steps [gidx*GS, (gidx+1)*GS) of direction s."""
        d = dnames[s]
        g = gbufs[s][(gidx * GS) // TC]
        col = ((gidx * GS) % TC) * B
        zq = zqpool[s].tile([128, 2 * GS * B], f32, tag="zq")
        if xf16:
            gf = g.bitcast(f16)
            gx = bass.AP(tensor=gf.tensor, offset=gf.offset + 2 * col,
                         ap=[[gf.ap[0][0], 65], [2, GS * B]])
        else:
            gx = g[0:65, col:col + GS * B]
        if kn.get("no_const"):
            nc.tensor.matmul(zq[:, 0:GS * B], W[f"x_if_{d}"], gx,
                             start=True, stop=False, skip_group_check=True)
            nc.tensor.matmul(zq[:, GS * B:2 * GS * B], W[f"x_og_{d}"], gx,
                             start=True, stop=False, skip_group_check=True)
        else:
            nc.tensor.matmul(zq[:, 0:GS * B], W[f"b_if_{d}"], ones_mv,
                             start=True, stop=False, skip_group_check=True)
            nc.tensor.matmul(zq[:, 0:GS * B], W[f"x_if_{d}"], gx,
                             start=False, stop=False, skip_group_check=True)
            nc.tensor.matmul(zq[:, GS * B:2 * GS * B], W[f"b_og_{d}"], ones_mv,
                             start=True, stop=False, skip_group_check=True)
            nc.tensor.matmul(zq[:, GS * B:2 * GS * B], W[f"x_og_{d}"], gx,
                             start=False, stop=False, skip_group_check=True)
        zqt[s][gidx] = zq

    issue_group(0, 0)
    issue_group(1, 0)


    NG = T // GS
    for n in range(T):
        c_ = n // TC
        if n % TC == 0 and c_ + 1 < NCH:
            issue_fetch(0, c_ + 1)
            issue_fetch(1, c_ + 1)
        for s in (0, 1):
            d = dnames[s]
            gidx, k = n // GS, n % GS
            zq = zqt[s][gidx]
            # h-part of step n's gates (h from step n-1; zeros at n=0)
            hmv = h_prev[s][0:64, :]
            nc.tensor.matmul(zq[:, k * B:(k + 1) * B], W[f"h_if_{d}"], hmv,
                             start=False, stop=True, skip_group_check=True)
            nc.tensor.matmul(zq[:, (GS + k) * B:(GS + k + 1) * B], W[f"h_og_{d}"], hmv,
                             start=False, stop=True, skip_group_check=True)


            # gates: sigma over [if | og] blocks of this step
            zv = bass.AP(tensor=zq.tensor, offset=zq.offset + k * B,
                         ap=[zq.ap[0], [GS * B, 2], [1, B]])
            S = spool.tile([128, 2 * B], f32, tag=f"S{s}")
            S3 = S.rearrange("p (n d) -> p n d", d=B)
            nc.scalar.activation(S3, zv, Sig)
            # i=S[0:64,0:B] f=S[64:128,0:B] sg=S[0:64,B:2B] o=S[64:128,B:2B]
            v = work.tile([128, B], f32, tag=f"v{s}")
            nc.vector.tensor_tensor(v[0:64, :], S[0:64, 0:B], S[0:64, B:2 * B], op=Op.mult)
            t3 = work.tile([128, B], f32, tag=f"t3{s}")
            nc.vector.scalar_tensor_tensor(
                t3[64:128, :], v[0:64, :], 2.0, S[0:64, 0:B], op0=Op.mult, op1=Op.subtract)
            u = work.tile([128, B], f32, tag=f"u{s}")
            ueng = nc.gpsimd if stage else nc.vector
            ueng.tensor_tensor(u[64:128, :], S[64:128, 0:B], cst[s][64:128, :], op=Op.mult)
            nc.vector.tensor_tensor(cst[s][64:128, :], t3[64:128, :], u[64:128, :], op=Op.add)
            tcn = work.tile([128, B], f16, tag=f"tc{s}")
            nc.scalar.activation(tcn[64:128, :], cst[s][64:128, :], Tanh)
            # o carry: overwrite o with previous o_sel where masked (1-ind != 0)
            g = gbufs[s][c_]
            gm = g if xf16 else g.bitcast(u32)
            mask = gm[64:128, (n % TC) * B:(n % TC) * B + B]
            nc.vector.copy_predicated(S[64:128, B:2 * B], mask, o_prev[s])
            o_prev[s] = S[64:128, B:2 * B]
            h = hpool.tile([128, B], f16, tag=f"h{s}")
            nc.vector.tensor_tensor(h[0:64, :], S[64:128, B:2 * B], tcn[64:128, :], op=Op.mult)
            h_prev[s] = h
            h_last[s] = h
            if k == GS - 1 and gidx + 1 < NG:
                issue_group(s, gidx + 1)

    # --- write out: out[b, s*64:(s+1)*64] = h_s[:, b]
    for s in range(2):
        hf = work.tile([128, B], f32, tag=f"hf{s}")
        nc.vector.tensor_copy(hf[0:64, :], h_last[s][0:64, :])
        dst = out[:, s * HID:(s + 1) * HID].transpose((1, 0))
        nc.sync.dma_start(out=dst, in_=hf[0:64, :])

    stack.close()


def _compile(T: int, knobs=None):
    import concourse.bacc as bacc
    import concourse.tile as tile
    from concourse import mybir

    key = (T, tuple(sorted((knobs or {}).items())))
    if key in _COMPILED:
        return _COMPILED[key]

    nc = bacc.Bacc("TRN2", num_devices=N_CORES)
    f32 = mybir.dt.float32
    f32r = mybir.dt.float32r
    f16 = mybir.dt.float16
    i16 = mybir.dt.int16

    ins = {}
    def din(name, shape, dtype):
        ins[name] = nc.dram_tensor(name, shape, dtype, kind="ExternalInput").ap()

    kn_ = dict(DEFAULT_KNOBS)
    kn_.update(knobs or {})
    if kn_.get("xf16"):
        din("tab16", [128, VOCAB], mybir.dt.uint32)
    else:
        din("tab", [128, VOCAB], f32 if kn_.get("no_f32r", 1) else f32r)
    din("idx16", [128, 2 * T * B // 16], i16)
    for d in ("f", "b"):
        if kn_.get("xf16"):
            din(f"lx16_if_{d}", [65, 128], f16)
            din(f"lx16_og_{d}", [65, 128], f16)
        else:
            din(f"lx_if_{d}", [65, 128], f32 if kn_.get("no_f32r", 1) else f32r)
            din(f"lx_og_{d}", [65, 128], f32 if kn_.get("no_f32r", 1) else f32r)
        din(f"lh_if_{d}", [64, 128], f16)
        din(f"lh_og_{d}", [64, 128], f16)
        din(f"bc_if_{d}", [1, 128], f16)
        din(f"bc_og_{d}", [1, 128], f16)
    out = nc.dram_tensor("out", [B, 2 * HID], f32, kind="ExternalOutput").ap()

    with tile.TileContext(nc) as tc:
        _build_body(tc, {"out": out}, ins, T=T, knobs=knobs)
    nc.compile()

    _COMPILED[key] = (nc, list(ins.keys()))
    return _COMPILED[key]


def kernel(tokens, emb_table, Wx_f, Wh_f, b_f, Wx_b, Wh_b, b_b):
    from concourse import bass_utils

    tokens = np.asarray(tokens)
    T = tokens.shape[1]
    nc, in_names = _compile(T)

    shared = _host_prep_shared(
        np.asarray(emb_table), np.asarray(Wx_f), np.asarray(Wh_f), np.asarray(b_f),
        np.asarray(Wx_b), np.asarray(Wh_b), np.asarray(b_b))

    in_maps = []
    for c in range(N_CORES):
        tok_c = tokens[c * B:(c + 1) * B]
        m = {k: v for k, v in shared.items() if k in in_names}
        m["idx16"] = _host_prep_core(tok_c, T)
        in_maps.append(m)

    res = bass_utils.run_bass_kernel_spmd(nc, in_maps, core_ids=list(range(N_CORES)))
    global _LAST_RESULTS, _LAST_EXEC_NS
    _LAST_RESULTS = res
    _LAST_EXEC_NS = getattr(res, "exec_time_ns", None)
    outs = [res.results[c]["out"] for c in range(N_CORES)]
    return np.concatenate(outs, axis=0).astype(np.float32)
